# revision 9
# baseline (speedup 1.0000x reference)
"""Trainium2 Bass kernel for the BiLSTM language-model head problem.

Strategy (8 NeuronCores):
  - The BiLSTM (embedding gather, xg precompute, 512-step forward+backward
    recurrence) is replicated on every core: its cost is dominated by
    streaming Wh through the PE each step, which is independent of batch,
    so data-parallelism would not help, and replication needs no
    cross-core communication.
  - The output projection (fc_w: [32000, 1024], and the 1 GB logits
    output) is sharded over the vocab dimension: core k computes
    logits[:, :, 4000k:4000k+4000] and writes ~134 MB.

Layout notes:
  - All matmul streams are bf16 (1 cycle/row on the PE); weights are
    converted to bf16 on the host so no on-device staging is needed.
    Gate weights/biases are pre-scaled by GSCALE on the host; the
    sigmoid/tanh activation instructions undo it via scale=1/GSCALE.
  - Recurrence matmuls are issued k-outer/j-inner so the four PE column
    groups (tile_position=(0,32j), M=16 each) stream concurrently
    (~216ns per 4-wide round of N=512 bf16 streams).
  - xg enters PSUM through a scatter-matrix inject matmul (iinj, 64->112
    partitions) that also opens the accumulation group, keeping the
    gate-sum off the DVE critical chain.
  - The two directions keep fully separate activation/cell-update
    chains so d0's chain latency hides under d1's matmul block and vice
    versa; PE emission is staggered (tr_d1(t-1) between the two MM
    blocks, tr_d0(t) after) so a transpose waiting on its hn never
    head-of-line blocks the opposite direction's matmuls.
  - fc phase: the hidden chunk is the stationary operand (one weight load
    per 4 matmuls), fc_w streams; logits come out row-major [R, VSP].
  - Phase A emits xg_f in ascending and xg_b in descending step order so
    the recurrence consumes both ends immediately.
Measured on trn2 (per core, NEFF exec): ~0.62ms gather+xg precompute,
~2.08ms recurrence (chain-latency bound), ~0.95ms fc = ~3.65ms total.
"""

import os
import sys

sys.path.insert(0, "/opt/trn_rl_repo")

import numpy as np
import ml_dtypes

BF = ml_dtypes.bfloat16
F8 = ml_dtypes.float8_e4m3fn
GSCALE = 512.0     # host-side gate-preactivation scale (fp8 range), undone
                   # by the activation instructions' scale=1/GSCALE

B = 16
T = 512
H = 512
G = 4 * H          # 2048
V = 32000
NCORES = 8
VS = V // NCORES   # 4000
VSP = 4096         # padded vocab shard (32 tiles of 128)
R = T * B          # 8192 rows, row = t*16 + b
CHUNT = 64         # recurrence steps per hiddenT spill chunk
NCH = T // CHUNT   # 8

_PROGRAM_CACHE = {}


def _split_excess_waits(nc, limit=1):
    """walrus codegen only tolerates a single sync wait on most
    instructions; move excess on_wait entries onto preceding NoOps."""
    import concourse.mybir as mybir

    n_split = 0
    for f in nc.m.functions:
        for blk in f.blocks:
            new_insts = []
            for inst in blk.instructions:
                si = inst.sync_info
                if si is not None and si.on_wait and len(si.on_wait) > limit:
                    waits = list(si.on_wait)
                    extra, keep = waits[:-limit], waits[-limit:]
                    for i in range(0, len(extra), limit):
                        chunk = extra[i:i + limit]
                        nop = mybir.InstNoOp(
                            name=nc.get_next_instruction_name(),
                            sync_info=mybir.SyncInfo(on_wait=chunk, on_update=[]),
                            bass_nofuse=True,
                            engine=inst.engine,
                        )
                        new_insts.append(nop)
                        n_split += 1
                    inst.sync_info = mybir.SyncInfo(
                        on_wait=keep, on_update=list(si.on_update or []))
                new_insts.append(inst)
            blk.instructions = new_insts
    return n_split


def _patch_tile_drain():
    """Split the TileContext tail drain's many sem waits into single-wait
    NoOps (same walrus limitation as above)."""
    import re

    import concourse.tile as tile
    from concourse.vector_clock import ScopedClock, VectorClock

    def _drain_and_barrier(self, tick_clock, wait_clock):
        gc = tick_clock.global_clock
        ticks = [int(x) for x in re.findall(r"\d+", repr(gc))]
        for i, t in enumerate(ticks):
            if t > 0:
                v = VectorClock()
                v.require_at_least(i, t)
                nop = self.nc.sync.nop(nofuse=True, hint=f"drain_split_{i}")
                wait_clock.add_sem_waits(nop.ins, ScopedClock({None: v}))
        self.nc.sync.drain()
        self.nc.all_engine_barrier()
        assert self.sems is not None
        popped = self.nc._tile_sem_poison_stack.pop()
        assert popped is self._sem_poison
        self.nc.clear_and_free_semaphores(list(self.sems.allocated().values()))
        self.nc.all_engine_barrier()

    tile.TileContext._drain_and_barrier = _drain_and_barrier


def _build_program():
    import concourse.bass as bass
    import concourse.mybir as mybir
    import concourse.tile as tile

    _patch_tile_drain()

    F32 = mybir.dt.float32
    BF16 = mybir.dt.bfloat16
    FP8 = mybir.dt.float8e4
    I32 = mybir.dt.int32
    AF = mybir.ActivationFunctionType
    OP = mybir.AluOpType

    nc = bass.Bass()

    # ---- I/O ----
    idx_d = nc.dram_tensor("idx", [128, R // 128], I32, kind="ExternalInput")
    emb_d = nc.dram_tensor("emb", [V, H], BF16, kind="ExternalInput")
    # input-side / recurrent weights, transposed + gate-permuted, both dirs
    wxtp_d = nc.dram_tensor("wxtp", [128, 2, 4, G], BF16, kind="ExternalInput")
    whtp_d = nc.dram_tensor("whtp", [128, 2, 4, G], BF16, kind="ExternalInput")
    # summed gate biases, replicated across partitions
    bsum_d = nc.dram_tensor("bsum", [128, 2, G], F32, kind="ExternalInput")
    i128_d = nc.dram_tensor("i128", [128, 128], BF16, kind="ExternalInput")
    iinj_d = nc.dram_tensor("iinj", [64, 112], BF16, kind="ExternalInput")
    # fc shard: [p, kk, vpad]  (kk<4 forward h-chunks, kk>=4 backward)
    fcwt_d = nc.dram_tensor("fcwt", [128, 8, VSP], BF16, kind="ExternalInput")
    fcbb_d = nc.dram_tensor("fcbb", [128, VSP], F32, kind="ExternalInput")

    logits_d = nc.dram_tensor("logits", [R, VSP], F32, kind="ExternalOutput")

    with tile.TileContext(nc) as tc:
        with (
            tc.tile_pool(name="const", bufs=1) as cpool,
            tc.tile_pool(name="dram", bufs=1, space="DRAM") as dpool,
        ):
            # DRAM scratch as pool tiles so Tile tracks cross-phase deps.
            # xg stored per step compactly: [t, gate-group j, batch, 512]
            xg_f_d = dpool.tile([T, 4, B, 512], BF16, tag="xg_f", name="xg_f")
            xg_b_d = dpool.tile([T, 4, B, 512], BF16, tag="xg_b", name="xg_b")
            ht_f_d = dpool.tile([NCH, 128, 4, B * CHUNT], BF16,
                                tag="ht_f", name="ht_f")
            ht_b_d = dpool.tile([NCH, 128, 4, B * CHUNT], BF16,
                                tag="ht_b", name="ht_b")
            i128 = cpool.tile([128, 128], BF16)
            nc.sync.dma_start(i128[:], i128_d[:])
            iinj = cpool.tile([64, 112], BF16)
            nc.sync.dma_start(iinj[:], iinj_d[:])

            # ================= PHASE A: gather + xg precompute =============
            with (
                tc.tile_pool(name="pa_sb", bufs=1) as pa,
                tc.tile_pool(name="pa_e", bufs=3) as pae,
                tc.tile_pool(name="pa_eT", bufs=3) as paet,
                tc.tile_pool(name="pa_xgo", bufs=6) as paxg,
                tc.tile_pool(name="pa_tr", bufs=2, space="PSUM") as patr,
                tc.tile_pool(name="pa_ps", bufs=3, space="PSUM") as paps,
            ):
                idx = pa.tile([128, R // 128], I32)
                nc.sync.dma_start(idx[:], idx_d[:])
                wxtp = pa.tile([128, 2, 4, G], BF16, tag="wxtp", name="wxtp")
                nc.sync.dma_start(wxtp[:], wxtp_d[:])
                bsum = pa.tile([128, 2, G], F32, tag="bsum", name="bsum")
                nc.sync.dma_start(bsum[:], bsum_d[:])

                for it in range(R // 128):
                    for d in range(2):
                        # xg_f tiles ascend, xg_b tiles descend so the
                        # recurrence (fwd reads t ascending, bwd reads
                        # T-1-t descending) can chase the precompute.
                        m = it if d == 0 else (R // 128 - 1 - it)
                        e_t = pae.tile([128, H], BF16, tag="e", name="e")
                        nc.gpsimd.indirect_dma_start(
                            out=e_t[:], out_offset=None, in_=emb_d[:],
                            in_offset=bass.IndirectOffsetOnAxis(
                                ap=idx[:, m:m + 1], axis=0))
                        ps_tr = patr.tile([128, 512], BF16, tag="pa_tr",
                                          name="pa_tr")
                        for k in range(4):
                            nc.tensor.matmul(
                                ps_tr[:, 128 * k:128 * k + 128],
                                e_t[:, 128 * k:128 * k + 128], i128[:],
                                is_transpose=True, start=True, stop=True,
                                skip_group_check=True)
                        eT = paet.tile([128, 512], BF16, tag="eT", name="eT")
                        nc.vector.tensor_copy(eT[:], ps_tr[:])

                        for jj in range(2):
                            px = [paps.tile([128, 512], F32, tag=f"px{j2}",
                                            name=f"px{j2}") for j2 in range(2)]
                            for k in range(4):
                                for j2 in range(2):
                                    j = 2 * jj + j2
                                    nc.tensor.matmul(
                                        px[j2][:],
                                        eT[:, 128 * k:128 * k + 128],
                                        wxtp[:, d, k, 512 * j:512 * j + 512],
                                        start=(k == 0), stop=(k == 3),
                                        skip_group_check=True)
                            for j2 in range(2):
                                j = 2 * jj + j2
                                xgo = paxg.tile([128, 512], BF16, tag="xgo",
                                                name="xgo")
                                nc.vector.tensor_tensor(
                                    xgo[:], px[j2][:],
                                    bsum[:, d, 512 * j:512 * j + 512],
                                    op=OP.add)
                                dst = xg_f_d if d == 0 else xg_b_d
                                nc.sync.dma_start(
                                    dst[8 * m:8 * m + 8, j, :, :], xgo[:])

            # ================= PHASE B: recurrence =========================
            # Per-direction chains are kept separate so d0's activation/cell
            # math overlaps d1's matmul streaming (and vice versa).  PE
            # emission order per step:
            #   inj0, MM0x16, [chain0 ops], tr1(t-1), inj1, MM1x16,
            #   [chain1 ops], tr0(t)
            # so a transpose never head-of-line blocks the opposite
            # direction's matmul block while its hn is still in flight.
            with (
                tc.tile_pool(name="pb_sb", bufs=1) as pb,
                tc.tile_pool(name="pb_xg", bufs=8) as pbx,
                tc.tile_pool(name="pb_gs", bufs=3) as pbg,
                tc.tile_pool(name="pb_tmp", bufs=3) as pbt,
                tc.tile_pool(name="pb_h", bufs=3) as pbh,
                tc.tile_pool(name="pb_acc", bufs=2) as pbacc,
                tc.tile_pool(name="pb_ps", bufs=2, space="PSUM") as pbps,
                tc.tile_pool(name="pb_ps2", bufs=2, space="PSUM") as pbps2,
            ):
                whtp = pb.tile([128, 2, 4, G], BF16, tag="whtp", name="whtp")
                nc.sync.dma_start(whtp[:], whtp_d[:])

                cst = pb.tile([128, 2, 128], F32, tag="cst", name="cst")
                nc.gpsimd.memset(cst[:], 0.0)
                hT0 = pbh.tile([128, 2, 128], BF16, tag="hT", name="hT")
                nc.gpsimd.memset(hT0[:], 0.0)

                def mm_block(d, t, hcur):
                    """inject xg then accumulate the recurrent matmuls."""
                    src = xg_f_d if d == 0 else xg_b_d
                    trow = t if d == 0 else T - 1 - t
                    xgt = pbx.tile([64, 512], BF16, tag=f"xgt{d}",
                                   name=f"xgt{d}")
                    nc.sync.dma_start(xgt[:], src[trow])
                    psg = pbps.tile([128, 512], F32, tag=f"psg{d}",
                                    name=f"psg{d}")
                    nc.tensor.matmul(psg[0:112, :], iinj[:], xgt[:],
                                     start=True, stop=False,
                                     skip_group_check=True)
                    for k in range(4):
                        for j in range(4):
                            nc.tensor.matmul(
                                psg[32 * j:32 * j + 16, :],
                                hcur[:, d, 32 * k:32 * k + 16],
                                whtp[:, d, k, 512 * j:512 * j + 512],
                                start=False, stop=(k == 3),
                                tile_position=(0, 32 * j),
                                skip_group_check=True)
                    return psg

                def chain(d, psg):
                    """sigmoid/tanh + cell update for one direction;
                    returns the new hidden state hn (gate layout)."""
                    gs = pbg.tile([128, 512], BF16, tag=f"gs{d}",
                                  name=f"gs{d}")
                    nc.scalar.activation(gs[0:112, 0:384], psg[0:112, 0:384],
                                         AF.Sigmoid, scale=1.0 / GSCALE)
                    nc.scalar.activation(gs[0:112, 384:512],
                                         psg[0:112, 384:512], AF.Tanh,
                                         scale=1.0 / GSCALE)
                    t1 = pbt.tile([128, 128], F32, tag=f"t1_{d}",
                                  name=f"t1_{d}")
                    nc.vector.tensor_tensor(
                        t1[0:112], gs[0:112, 0:128], cst[0:112, d],
                        op=OP.mult)
                    t2 = pbt.tile([128, 128], BF16, tag=f"t2_{d}",
                                  name=f"t2_{d}")
                    nc.vector.tensor_tensor(
                        t2[0:112], gs[0:112, 128:256], gs[0:112, 384:512],
                        op=OP.mult)
                    nc.vector.tensor_tensor(
                        cst[0:112, d], t1[0:112], t2[0:112], op=OP.add)
                    tcn = pbt.tile([128, 128], BF16, tag=f"tcn{d}",
                                   name=f"tcn{d}")
                    nc.scalar.activation(tcn[0:112], cst[0:112, d], AF.Tanh)
                    hn = pbt.tile([128, 128], BF16, tag=f"hn{d}",
                                  name=f"hn{d}")
                    nc.vector.tensor_tensor(
                        hn[0:112], gs[0:112, 256:384], tcn[0:112],
                        op=OP.mult)
                    return hn

                def transpose_h(d, t, hn, hdst, acc):
                    """hn -> hT slice (PE transpose + DVE evacuation), plus
                    the time-ordered hidden accumulator copy on GpSimd."""
                    pst = pbps2.tile([128, 128], BF16, tag=f"pst{d}",
                                     name=f"pst{d}")
                    nc.tensor.matmul(pst[:], hn[:], i128[:],
                                     is_transpose=True, start=True,
                                     stop=True, skip_group_check=True)
                    nc.vector.tensor_copy(hdst[:, d, :], pst[:])
                    tcc = t % CHUNT
                    pos = tcc if d == 0 else CHUNT - 1 - tcc
                    nc.gpsimd.tensor_copy(
                        acc[:, d, :, B * pos:B * pos + B],
                        hdst[:, d].rearrange("p (j b) -> p j b",
                                             b=32)[:, :, 0:16])

                hcur = hT0
                hn_prev1 = None     # d1's hn from step t-1
                acc = None
                accprev = None
                for t in range(T):
                    tcc = t % CHUNT
                    if tcc == 0:
                        accprev = acc
                        acc = pbacc.tile([128, 2, 4, B * CHUNT], BF16,
                                         tag="acc", name="acc")
                    psg0 = mm_block(0, t, hcur)
                    hn0 = chain(0, psg0)
                    if t > 0:
                        # d1 transpose of the previous step lands into the
                        # previous step's hT tile, just before MM1 reads it.
                        a1 = acc if tcc != 0 else accprev
                        transpose_h(1, t - 1, hn_prev1, hcur, a1)
                        if tcc == 0:
                            cb = t // CHUNT - 1
                            nc.sync.dma_start(ht_b_d[NCH - 1 - cb],
                                              accprev[:, 1])
                    psg1 = mm_block(1, t, hcur)
                    hn_prev1 = chain(1, psg1)
                    hnext = pbh.tile([128, 2, 128], BF16, tag="hT",
                                     name="hT")
                    transpose_h(0, t, hn0, hnext, acc)
                    if tcc == CHUNT - 1:
                        nc.sync.dma_start(ht_f_d[t // CHUNT], acc[:, 0])
                    hcur = hnext
                # drain the final backward-direction step
                transpose_h(1, T - 1, hn_prev1, hcur, acc)
                nc.sync.dma_start(ht_b_d[0], acc[:, 1])

            # ================= PHASE C: fc =================================
            with (
                tc.tile_pool(name="pc_sb", bufs=1) as pc,
                tc.tile_pool(name="pc_h", bufs=3) as pch,
                tc.tile_pool(name="pc_o", bufs=6) as pco,
                tc.tile_pool(name="pc_ps", bufs=2, space="PSUM") as pcps,
            ):
                fcw = pc.tile([128, 8, VSP], BF16, tag="fcw", name="fcw")
                nc.sync.dma_start(fcw[:], fcwt_d[:])
                fcbb = pc.tile([128, VSP], F32, tag="fcbb", name="fcbb")
                nc.sync.dma_start(fcbb[:], fcbb_d[:])

                for rb in range(R // 128):
                    c64 = rb // 8
                    off = 128 * (rb % 8)
                    hTt = pch.tile([128, 8, 128], BF16, tag="hTt", name="hTt")
                    nc.sync.dma_start(hTt[:, 0:4, :],
                                      ht_f_d[c64][:, :, off:off + 128])
                    nc.sync.dma_start(hTt[:, 4:8, :],
                                      ht_b_d[c64][:, :, off:off + 128])
                    for half in range(2):
                        psl = [pcps.tile([128, 512], F32, tag=f"psl{v}",
                                         name=f"psl{v}") for v in range(4)]
                        for kk in range(8):
                            for v in range(4):
                                vv = 4 * half + v
                                nc.tensor.matmul(
                                    psl[v][:], hTt[:, kk, :],
                                    fcw[:, kk, 512 * vv:512 * vv + 512],
                                    start=(kk == 0), stop=(kk == 7),
                                    skip_group_check=True)
                        for v in range(4):
                            vv = 4 * half + v
                            lo = pco.tile([128, 512], F32, tag="lo",
                                          name="lo")
                            nc.vector.tensor_tensor(
                                lo[:], psl[v][:],
                                fcbb[:, 512 * vv:512 * vv + 512], op=OP.add)
                            nc.sync.dma_start(
                                logits_d[128 * rb:128 * rb + 128,
                                         512 * vv:512 * vv + 512], lo[:])

    _split_excess_waits(nc)
    return nc


def _host_prep(x, emb, Wf_x, bf_x, Wf_h, bf_h, Wb_x, bb_x, Wb_h, bb_h,
               fc_w, fc_b):
    """Build per-core input maps (host-side layout shuffles only)."""
    # gate permutation: permuted col 512j + 128q + d  <-  orig gate q' h-dim
    # 128j + d, chunk-internal gate order [f, i, o, ct]
    orig_off = [0, 512, 1536, 1024]  # f, i, o, ct
    perm = np.zeros(G, np.int64)
    for j in range(4):
        for q in range(4):
            perm[512 * j + 128 * q:512 * j + 128 * q + 128] = (
                orig_off[q] + 128 * j + np.arange(128))

    def wT_perm(w):   # [G, H] -> [128, 4, G] permuted transposed
        wt = np.ascontiguousarray(w.T)[:, perm]          # [H, G]
        return np.ascontiguousarray(
            wt.reshape(4, 128, G).transpose(1, 0, 2))

    wxtp = (np.stack([wT_perm(Wf_x), wT_perm(Wb_x)], axis=1)
            * GSCALE).astype(BF)
    whtp = (np.stack([wT_perm(Wf_h), wT_perm(Wb_h)], axis=1)
            * GSCALE).astype(BF)
    bsum2 = np.stack([(bf_x + bf_h)[perm], (bb_x + bb_h)[perm]]) * GSCALE
    bsum = np.ascontiguousarray(
        np.broadcast_to(bsum2[None], (128, 2, G)), dtype=np.float32)

    # token index tile: idx[p, m] = x[b, t] with t*16+b = 128m + p
    xT = np.ascontiguousarray(x.T.astype(np.int32)).reshape(R)  # row t*16+b
    idx = np.ascontiguousarray(xT.reshape(R // 128, 128).T)

    i128 = np.eye(128, dtype=BF)
    iinj = np.zeros((64, 112), np.float32)
    for j in range(4):
        for b in range(B):
            iinj[16 * j + b, 32 * j + b] = 1.0
    iinj = iinj.astype(BF)
    embb = np.asarray(emb, np.float32).astype(BF)

    base = {
        "idx": idx, "emb": embb,
        "wxtp": wxtp, "whtp": whtp, "bsum": bsum,
        "i128": i128, "iinj": iinj,
    }

    in_maps = []
    for core in range(NCORES):
        w = fc_w[VS * core:VS * core + VS]               # [4000, 1024]
        wpad = np.zeros((VSP, 2 * H), np.float32)
        wpad[:VS] = w
        fcwt = np.ascontiguousarray(
            wpad.T.reshape(8, 128, VSP).transpose(1, 0, 2)).astype(BF)
        bpad = np.zeros(VSP, np.float32)
        bpad[:VS] = fc_b[VS * core:VS * core + VS]
        fcbb = np.ascontiguousarray(
            np.broadcast_to(bpad[None], (128, VSP)), dtype=np.float32)
        m = dict(base)
        m["fcwt"] = fcwt
        m["fcbb"] = fcbb
        in_maps.append(m)
    return in_maps


def kernel(x, emb, Wf_x, bf_x, Wf_h, bf_h, Wb_x, bb_x, Wb_h, bb_h,
           fc_w, fc_b, _return_raw=False):
    from concourse.bass_utils import run_bass_kernel_spmd

    args = [np.asarray(a) for a in (
        x, emb, Wf_x, bf_x, Wf_h, bf_h, Wb_x, bb_x, Wb_h, bb_h, fc_w, fc_b)]
    x = args[0]

    key = "prog"
    if key not in _PROGRAM_CACHE:
        _PROGRAM_CACHE[key] = _build_program()
    nc = _PROGRAM_CACHE[key]

    in_maps = _host_prep(*args)
    res = run_bass_kernel_spmd(nc, in_maps, list(range(NCORES)))

    out = np.empty((B, T, V), np.float32)
    for core in range(NCORES):
        lt = res.results[core]["logits"]                 # [8192, 4096]
        out[:, :, VS * core:VS * core + VS] = (
            lt.reshape(T, B, VSP)[:, :, :VS].transpose(1, 0, 2))
    if _return_raw:
        return out, res
    return out


# revision 10
# speedup vs baseline: 1.1936x; 1.1936x over previous
"""Trainium2 Bass kernel for the BiLSTM language-model head problem.

Strategy (8 NeuronCores):
  - The BiLSTM (embedding gather, xg precompute, 512-step forward+backward
    recurrence) is replicated on every core: its cost is dominated by
    streaming Wh through the PE each step, which is independent of batch,
    so data-parallelism would not help, and replication needs no
    cross-core communication.
  - The output projection (fc_w: [32000, 1024], and the 1 GB logits
    output) is sharded over the vocab dimension: core k computes
    logits[:, :, 4000k:4000k+4000] and writes ~134 MB.

Layout notes:
  - All matmul streams are bf16 (1 cycle/row on the PE); weights are
    converted to bf16 on the host so no on-device staging is needed.
    Gate weights/biases are pre-scaled by GSCALE on the host; the
    sigmoid/tanh activation instructions undo it via scale=1/GSCALE.
  - Recurrence matmuls are issued k-outer/j-inner so the four PE column
    groups (tile_position=(0,32j), M=16 each) stream concurrently
    (~216ns per 4-wide round of N=512 bf16 streams).
  - xg enters PSUM through a scatter-matrix inject matmul (iinj, 64->112
    partitions) that also opens the accumulation group, keeping the
    gate-sum off the DVE critical chain.
  - The two directions keep fully separate activation/cell-update
    chains so d0's chain latency hides under d1's matmul block and vice
    versa; PE emission is staggered (tr_d1(t-1) between the two MM
    blocks, tr_d0(t) after) so a transpose waiting on its hn never
    head-of-line blocks the opposite direction's matmuls.
  - fc phase: the hidden chunk is the stationary operand (one weight load
    per 4 matmuls), fc_w streams; logits come out row-major [R, VSP].
  - Phase A emits xg_f in ascending and xg_b in descending step order so
    the recurrence consumes both ends immediately.
Measured on trn2 (per core, NEFF exec): ~0.62ms gather+xg precompute,
~2.08ms recurrence (chain-latency bound), ~0.95ms fc = ~3.65ms total.
"""

import os
import sys

sys.path.insert(0, "/opt/trn_rl_repo")

import numpy as np
import ml_dtypes

BF = ml_dtypes.bfloat16
F8 = ml_dtypes.float8_e4m3fn
GSCALE = 512.0     # host-side gate-preactivation scale (fp8 range), undone
                   # by the activation instructions' scale=1/GSCALE

B = 16
T = 512
H = 512
G = 4 * H          # 2048
V = 32000
NCORES = 8
VS = V // NCORES   # 4000
VSP = 4096         # padded vocab shard (32 tiles of 128)
R = T * B          # 8192 rows, row = t*16 + b
CHUNT = 64         # recurrence steps per hiddenT spill chunk
NCH = T // CHUNT   # 8

_PROGRAM_CACHE = {}


def _split_excess_waits(nc, limit=1):
    """walrus codegen only tolerates a single sync wait on most
    instructions; move excess on_wait entries onto preceding NoOps."""
    import concourse.mybir as mybir

    n_split = 0
    for f in nc.m.functions:
        for blk in f.blocks:
            new_insts = []
            for inst in blk.instructions:
                si = inst.sync_info
                if si is not None and si.on_wait and len(si.on_wait) > limit:
                    waits = list(si.on_wait)
                    extra, keep = waits[:-limit], waits[-limit:]
                    for i in range(0, len(extra), limit):
                        chunk = extra[i:i + limit]
                        nop = mybir.InstNoOp(
                            name=nc.get_next_instruction_name(),
                            sync_info=mybir.SyncInfo(on_wait=chunk, on_update=[]),
                            bass_nofuse=True,
                            engine=inst.engine,
                        )
                        new_insts.append(nop)
                        n_split += 1
                    inst.sync_info = mybir.SyncInfo(
                        on_wait=keep, on_update=list(si.on_update or []))
                new_insts.append(inst)
            blk.instructions = new_insts
    return n_split


def _patch_tile_drain():
    """Split the TileContext tail drain's many sem waits into single-wait
    NoOps (same walrus limitation as above)."""
    import re

    import concourse.tile as tile
    from concourse.vector_clock import ScopedClock, VectorClock

    def _drain_and_barrier(self, tick_clock, wait_clock):
        gc = tick_clock.global_clock
        ticks = [int(x) for x in re.findall(r"\d+", repr(gc))]
        for i, t in enumerate(ticks):
            if t > 0:
                v = VectorClock()
                v.require_at_least(i, t)
                nop = self.nc.sync.nop(nofuse=True, hint=f"drain_split_{i}")
                wait_clock.add_sem_waits(nop.ins, ScopedClock({None: v}))
        self.nc.sync.drain()
        self.nc.all_engine_barrier()
        assert self.sems is not None
        popped = self.nc._tile_sem_poison_stack.pop()
        assert popped is self._sem_poison
        self.nc.clear_and_free_semaphores(list(self.sems.allocated().values()))
        self.nc.all_engine_barrier()

    tile.TileContext._drain_and_barrier = _drain_and_barrier


def _build_program():
    import concourse.bass as bass
    import concourse.mybir as mybir
    import concourse.tile as tile

    _patch_tile_drain()

    F32 = mybir.dt.float32
    BF16 = mybir.dt.bfloat16
    FP8 = mybir.dt.float8e4
    I32 = mybir.dt.int32
    AF = mybir.ActivationFunctionType
    OP = mybir.AluOpType

    nc = bass.Bass()

    # ---- I/O ----
    idx_d = nc.dram_tensor("idx", [128, R // 128], I32, kind="ExternalInput")
    emb_d = nc.dram_tensor("emb", [V, H], BF16, kind="ExternalInput")
    # input-side / recurrent weights, transposed + gate-permuted, both dirs
    wxtp_d = nc.dram_tensor("wxtp", [128, 2, 4, G], BF16, kind="ExternalInput")
    whtp_d = nc.dram_tensor("whtp", [128, 2, 4, G], BF16, kind="ExternalInput")
    # summed gate biases, replicated across partitions
    bsum_d = nc.dram_tensor("bsum", [128, 2, G], F32, kind="ExternalInput")
    i128_d = nc.dram_tensor("i128", [128, 128], BF16, kind="ExternalInput")
    iinj_d = nc.dram_tensor("iinj", [64, 112], BF16, kind="ExternalInput")
    # fc shard: [p, kk, vpad]  (kk<4 forward h-chunks, kk>=4 backward)
    fcwt_d = nc.dram_tensor("fcwt", [128, 8, VSP], BF16, kind="ExternalInput")
    fcbb_d = nc.dram_tensor("fcbb", [128, VSP], F32, kind="ExternalInput")

    logits_d = nc.dram_tensor("logits", [R, VSP], F32, kind="ExternalOutput")

    with tile.TileContext(nc) as tc:
        with (
            tc.tile_pool(name="const", bufs=1) as cpool,
            tc.tile_pool(name="dram", bufs=1, space="DRAM") as dpool,
        ):
            # DRAM scratch as pool tiles so Tile tracks cross-phase deps.
            # xg stored per step compactly: [t, gate-group j, batch, 512]
            xg_f_d = dpool.tile([T, 4, B, 512], BF16, tag="xg_f", name="xg_f")
            xg_b_d = dpool.tile([T, 4, B, 512], BF16, tag="xg_b", name="xg_b")
            ht_f_d = dpool.tile([NCH, 128, 4, B * CHUNT], BF16,
                                tag="ht_f", name="ht_f")
            ht_b_d = dpool.tile([NCH, 128, 4, B * CHUNT], BF16,
                                tag="ht_b", name="ht_b")
            i128 = cpool.tile([128, 128], BF16)
            nc.sync.dma_start(i128[:], i128_d[:])
            iinj = cpool.tile([64, 112], BF16)
            nc.sync.dma_start(iinj[:], iinj_d[:])

            # ================= PHASE A: gather + xg precompute =============
            with (
                tc.tile_pool(name="pa_sb", bufs=1) as pa,
                tc.tile_pool(name="pa_e", bufs=4) as pae,
                tc.tile_pool(name="pa_eT", bufs=4) as paet,
                tc.tile_pool(name="pa_xgo", bufs=6) as paxg,
                tc.tile_pool(name="pa_tr", bufs=2, space="PSUM") as patr,
                tc.tile_pool(name="pa_ps", bufs=3, space="PSUM") as paps,
            ):
                idx = pa.tile([128, R // 128], I32)
                nc.sync.dma_start(idx[:], idx_d[:])
                wxtp = pa.tile([128, 2, 4, G], BF16, tag="wxtp", name="wxtp")
                nc.sync.dma_start(wxtp[:], wxtp_d[:])
                bsum = pa.tile([128, 2, G], F32, tag="bsum", name="bsum")
                nc.sync.dma_start(bsum[:], bsum_d[:])

                for it in range(R // 128):
                    for d in range(2):
                        # xg_f tiles ascend, xg_b tiles descend so the
                        # recurrence (fwd reads t ascending, bwd reads
                        # T-1-t descending) can chase the precompute.
                        m = it if d == 0 else (R // 128 - 1 - it)
                        e_t = pae.tile([128, H], BF16, tag="e", name="e")
                        nc.gpsimd.indirect_dma_start(
                            out=e_t[:], out_offset=None, in_=emb_d[:],
                            in_offset=bass.IndirectOffsetOnAxis(
                                ap=idx[:, m:m + 1], axis=0))
                        ps_tr = patr.tile([128, 512], BF16, tag="pa_tr",
                                          name="pa_tr")
                        for k in range(4):
                            nc.tensor.matmul(
                                ps_tr[:, 128 * k:128 * k + 128],
                                e_t[:, 128 * k:128 * k + 128], i128[:],
                                is_transpose=True, start=True, stop=True,
                                skip_group_check=True)
                        eT = paet.tile([128, 512], BF16, tag="eT", name="eT")
                        nc.vector.tensor_copy(eT[:], ps_tr[:])

                        for jj in range(2):
                            px = [paps.tile([128, 512], F32, tag=f"px{j2}",
                                            name=f"px{j2}") for j2 in range(2)]
                            for k in range(4):
                                for j2 in range(2):
                                    j = 2 * jj + j2
                                    nc.tensor.matmul(
                                        px[j2][:],
                                        eT[:, 128 * k:128 * k + 128],
                                        wxtp[:, d, k, 512 * j:512 * j + 512],
                                        start=(k == 0), stop=(k == 3),
                                        skip_group_check=True)
                            for j2 in range(2):
                                j = 2 * jj + j2
                                xgo = paxg.tile([128, 512], BF16, tag="xgo",
                                                name="xgo")
                                nc.vector.tensor_tensor(
                                    xgo[:], px[j2][:],
                                    bsum[:, d, 512 * j:512 * j + 512],
                                    op=OP.add)
                                dst = xg_f_d if d == 0 else xg_b_d
                                nc.sync.dma_start(
                                    dst[8 * m:8 * m + 8, j, :, :], xgo[:])

            # ================= PHASE B: recurrence =========================
            # Per-direction chains are kept separate so d0's activation/cell
            # math overlaps d1's matmul streaming (and vice versa).  PE
            # emission order per step:
            #   inj0, MM0x16, [chain0 ops], tr1(t-1), inj1, MM1x16,
            #   [chain1 ops], tr0(t)
            # so a transpose never head-of-line blocks the opposite
            # direction's matmul block while its hn is still in flight.
            with (
                tc.tile_pool(name="pb_sb", bufs=1) as pb,
                tc.tile_pool(name="pb_xg", bufs=12) as pbx,
                tc.tile_pool(name="pb_gs", bufs=4) as pbg,
                tc.tile_pool(name="pb_tmp", bufs=4) as pbt,
                tc.tile_pool(name="pb_h", bufs=4) as pbh,
                tc.tile_pool(name="pb_acc", bufs=2) as pbacc,
                tc.tile_pool(name="pb_ps", bufs=2, space="PSUM") as pbps,
                tc.tile_pool(name="pb_ps2", bufs=2, space="PSUM") as pbps2,
            ):
                whtp = pb.tile([128, 2, 4, G], BF16, tag="whtp", name="whtp")
                nc.sync.dma_start(whtp[:], whtp_d[:])

                cst = pb.tile([128, 2, 128], F32, tag="cst", name="cst")
                nc.gpsimd.memset(cst[:], 0.0)
                hT0 = pbh.tile([128, 2, 128], BF16, tag="hT", name="hT")
                nc.gpsimd.memset(hT0[:], 0.0)

                def mm_block(d, t, hcur):
                    """inject xg then accumulate the recurrent matmuls."""
                    src = xg_f_d if d == 0 else xg_b_d
                    trow = t if d == 0 else T - 1 - t
                    xgt = pbx.tile([64, 512], BF16, tag=f"xgt{d}",
                                   name=f"xgt{d}")
                    nc.sync.dma_start(xgt[:], src[trow])
                    psg = pbps.tile([128, 512], F32, tag=f"psg{d}",
                                    name=f"psg{d}")
                    nc.tensor.matmul(psg[0:112, :], iinj[:], xgt[:],
                                     start=True, stop=False,
                                     skip_group_check=True)
                    for k in range(4):
                        for j in range(4):
                            nc.tensor.matmul(
                                psg[32 * j:32 * j + 16, :],
                                hcur[:, d, 32 * k:32 * k + 16],
                                whtp[:, d, k, 512 * j:512 * j + 512],
                                start=False, stop=(k == 3),
                                tile_position=(0, 32 * j),
                                skip_group_check=True)
                    return psg

                def chain(d, psg):
                    """sigmoid/tanh + cell update for one direction;
                    returns the new hidden state hn (gate layout)."""
                    gs = pbg.tile([128, 512], BF16, tag=f"gs{d}",
                                  name=f"gs{d}")
                    nc.scalar.activation(gs[0:112, 0:384], psg[0:112, 0:384],
                                         AF.Sigmoid, scale=1.0 / GSCALE)
                    nc.scalar.activation(gs[0:112, 384:512],
                                         psg[0:112, 384:512], AF.Tanh,
                                         scale=1.0 / GSCALE)
                    t1 = pbt.tile([128, 128], F32, tag=f"t1_{d}",
                                  name=f"t1_{d}")
                    nc.vector.tensor_tensor(
                        t1[0:112], gs[0:112, 0:128], cst[0:112, d],
                        op=OP.mult)
                    t2 = pbt.tile([128, 128], BF16, tag=f"t2_{d}",
                                  name=f"t2_{d}")
                    nc.vector.tensor_tensor(
                        t2[0:112], gs[0:112, 128:256], gs[0:112, 384:512],
                        op=OP.mult)
                    nc.vector.tensor_tensor(
                        cst[0:112, d], t1[0:112], t2[0:112], op=OP.add)
                    tcn = pbt.tile([128, 128], BF16, tag=f"tcn{d}",
                                   name=f"tcn{d}")
                    nc.scalar.activation(tcn[0:112], cst[0:112, d], AF.Tanh)
                    hn = pbt.tile([128, 128], BF16, tag=f"hn{d}",
                                  name=f"hn{d}")
                    nc.vector.tensor_tensor(
                        hn[0:112], gs[0:112, 256:384], tcn[0:112],
                        op=OP.mult)
                    return hn

                def transpose_h(d, t, hn, hdst, acc):
                    """hn -> hT slice (PE transpose + DVE evacuation), plus
                    the time-ordered hidden accumulator copy on GpSimd."""
                    pst = pbps2.tile([128, 128], BF16, tag=f"pst{d}",
                                     name=f"pst{d}")
                    nc.tensor.matmul(pst[:], hn[:], i128[:],
                                     is_transpose=True, start=True,
                                     stop=True, skip_group_check=True)
                    nc.vector.tensor_copy(hdst[:, d, :], pst[:])
                    tcc = t % CHUNT
                    pos = tcc if d == 0 else CHUNT - 1 - tcc
                    nc.gpsimd.tensor_copy(
                        acc[:, d, :, B * pos:B * pos + B],
                        hdst[:, d].rearrange("p (j b) -> p j b",
                                             b=32)[:, :, 0:16])

                hcur = hT0
                hn_prev1 = None     # d1's hn from step t-1
                acc = None
                accprev = None
                for t in range(T):
                    tcc = t % CHUNT
                    if tcc == 0:
                        accprev = acc
                        acc = pbacc.tile([128, 2, 4, B * CHUNT], BF16,
                                         tag="acc", name="acc")
                    psg0 = mm_block(0, t, hcur)
                    hn0 = chain(0, psg0)
                    if t > 0:
                        # d1 transpose of the previous step lands into the
                        # previous step's hT tile, just before MM1 reads it.
                        a1 = acc if tcc != 0 else accprev
                        transpose_h(1, t - 1, hn_prev1, hcur, a1)
                        if tcc == 0:
                            cb = t // CHUNT - 1
                            nc.sync.dma_start(ht_b_d[NCH - 1 - cb],
                                              accprev[:, 1])
                    psg1 = mm_block(1, t, hcur)
                    hn_prev1 = chain(1, psg1)
                    hnext = pbh.tile([128, 2, 128], BF16, tag="hT",
                                     name="hT")
                    transpose_h(0, t, hn0, hnext, acc)
                    if tcc == CHUNT - 1:
                        nc.sync.dma_start(ht_f_d[t // CHUNT], acc[:, 0])
                    hcur = hnext
                # drain the final backward-direction step
                transpose_h(1, T - 1, hn_prev1, hcur, acc)
                nc.sync.dma_start(ht_b_d[0], acc[:, 1])

            # ================= PHASE C: fc =================================
            with (
                tc.tile_pool(name="pc_sb", bufs=1) as pc,
                tc.tile_pool(name="pc_h", bufs=4) as pch,
                tc.tile_pool(name="pc_o", bufs=8) as pco,
                tc.tile_pool(name="pc_ps", bufs=2, space="PSUM") as pcps,
            ):
                fcw = pc.tile([128, 8, VSP], BF16, tag="fcw", name="fcw")
                nc.sync.dma_start(fcw[:], fcwt_d[:])
                fcbb = pc.tile([128, VSP], F32, tag="fcbb", name="fcbb")
                nc.sync.dma_start(fcbb[:], fcbb_d[:])

                for rb in range(R // 128):
                    c64 = rb // 8
                    off = 128 * (rb % 8)
                    hTt = pch.tile([128, 8, 128], BF16, tag="hTt", name="hTt")
                    nc.sync.dma_start(hTt[:, 0:4, :],
                                      ht_f_d[c64][:, :, off:off + 128])
                    nc.sync.dma_start(hTt[:, 4:8, :],
                                      ht_b_d[c64][:, :, off:off + 128])
                    for half in range(2):
                        psl = [pcps.tile([128, 512], F32, tag=f"psl{v}",
                                         name=f"psl{v}") for v in range(4)]
                        for kk in range(8):
                            for v in range(4):
                                vv = 4 * half + v
                                nc.tensor.matmul(
                                    psl[v][:], hTt[:, kk, :],
                                    fcw[:, kk, 512 * vv:512 * vv + 512],
                                    start=(kk == 0), stop=(kk == 7),
                                    skip_group_check=True)
                        for v in range(4):
                            vv = 4 * half + v
                            lo = pco.tile([128, 512], F32, tag="lo",
                                          name="lo")
                            nc.vector.tensor_tensor(
                                lo[:], psl[v][:],
                                fcbb[:, 512 * vv:512 * vv + 512], op=OP.add)
                            nc.sync.dma_start(
                                logits_d[128 * rb:128 * rb + 128,
                                         512 * vv:512 * vv + 512], lo[:])

    _split_excess_waits(nc)
    return nc


def _host_prep(x, emb, Wf_x, bf_x, Wf_h, bf_h, Wb_x, bb_x, Wb_h, bb_h,
               fc_w, fc_b):
    """Build per-core input maps (host-side layout shuffles only)."""
    # gate permutation: permuted col 512j + 128q + d  <-  orig gate q' h-dim
    # 128j + d, chunk-internal gate order [f, i, o, ct]
    orig_off = [0, 512, 1536, 1024]  # f, i, o, ct
    perm = np.zeros(G, np.int64)
    for j in range(4):
        for q in range(4):
            perm[512 * j + 128 * q:512 * j + 128 * q + 128] = (
                orig_off[q] + 128 * j + np.arange(128))

    def wT_perm(w):   # [G, H] -> [128, 4, G] permuted transposed
        wt = np.ascontiguousarray(w.T)[:, perm]          # [H, G]
        return np.ascontiguousarray(
            wt.reshape(4, 128, G).transpose(1, 0, 2))

    wxtp = (np.stack([wT_perm(Wf_x), wT_perm(Wb_x)], axis=1)
            * GSCALE).astype(BF)
    whtp = (np.stack([wT_perm(Wf_h), wT_perm(Wb_h)], axis=1)
            * GSCALE).astype(BF)
    bsum2 = np.stack([(bf_x + bf_h)[perm], (bb_x + bb_h)[perm]]) * GSCALE
    bsum = np.ascontiguousarray(
        np.broadcast_to(bsum2[None], (128, 2, G)), dtype=np.float32)

    # token index tile: idx[p, m] = x[b, t] with t*16+b = 128m + p
    xT = np.ascontiguousarray(x.T.astype(np.int32)).reshape(R)  # row t*16+b
    idx = np.ascontiguousarray(xT.reshape(R // 128, 128).T)

    i128 = np.eye(128, dtype=BF)
    iinj = np.zeros((64, 112), np.float32)
    for j in range(4):
        for b in range(B):
            iinj[16 * j + b, 32 * j + b] = 1.0
    iinj = iinj.astype(BF)
    embb = np.asarray(emb, np.float32).astype(BF)

    base = {
        "idx": idx, "emb": embb,
        "wxtp": wxtp, "whtp": whtp, "bsum": bsum,
        "i128": i128, "iinj": iinj,
    }

    in_maps = []
    for core in range(NCORES):
        w = fc_w[VS * core:VS * core + VS]               # [4000, 1024]
        wpad = np.zeros((VSP, 2 * H), np.float32)
        wpad[:VS] = w
        fcwt = np.ascontiguousarray(
            wpad.T.reshape(8, 128, VSP).transpose(1, 0, 2)).astype(BF)
        bpad = np.zeros(VSP, np.float32)
        bpad[:VS] = fc_b[VS * core:VS * core + VS]
        fcbb = np.ascontiguousarray(
            np.broadcast_to(bpad[None], (128, VSP)), dtype=np.float32)
        m = dict(base)
        m["fcwt"] = fcwt
        m["fcbb"] = fcbb
        in_maps.append(m)
    return in_maps


def kernel(x, emb, Wf_x, bf_x, Wf_h, bf_h, Wb_x, bb_x, Wb_h, bb_h,
           fc_w, fc_b, _return_raw=False):
    from concourse.bass_utils import run_bass_kernel_spmd

    args = [np.asarray(a) for a in (
        x, emb, Wf_x, bf_x, Wf_h, bf_h, Wb_x, bb_x, Wb_h, bb_h, fc_w, fc_b)]
    x = args[0]

    key = "prog"
    if key not in _PROGRAM_CACHE:
        _PROGRAM_CACHE[key] = _build_program()
    nc = _PROGRAM_CACHE[key]

    in_maps = _host_prep(*args)
    res = run_bass_kernel_spmd(nc, in_maps, list(range(NCORES)))

    out = np.empty((B, T, V), np.float32)
    for core in range(NCORES):
        lt = res.results[core]["logits"]                 # [8192, 4096]
        out[:, :, VS * core:VS * core + VS] = (
            lt.reshape(T, B, VSP)[:, :, :VS].transpose(1, 0, 2))
    if _return_raw:
        return out, res
    return out
